# revision 1
# baseline (speedup 1.0000x reference)
import numpy as np
import jax
import jax.numpy as jnp

# nn_GatedElementBasedRNNLayer_Loop — hardcoded problem shapes
P, Q, B, I, H = 400, 50, 128, 256, 128
NC = 8
BS = B // NC  # per-core batch shard


def _forward(passage_repr, question_repr, WuQ, WuP, WvP, vT, Wg, W_ih, W_hh):
    h = H
    question = jnp.einsum('qbi,hi->qbh', question_repr, WuQ)   # (Q,b,H)
    passage = jnp.einsum('pbi,hi->pbh', passage_repr, WuP)     # (P,b,H)
    v = vT[0]                                                  # (H,)

    def step(prev, p_i):
        last = prev @ WvP.T                                    # (b,H)
        s = jnp.tanh(p_i[None, :, :] + question + last[None, :, :])
        logits = jnp.einsum('qbh,h->qb', s, v)                 # (Q,b)
        ai = jax.nn.softmax(logits, axis=0)[..., None]         # (Q,b,1)
        ct = jnp.sum(question * ai, axis=0)                    # (b,H)
        u = jnp.concatenate([p_i, ct], axis=-1)                # (b,2H)
        u = u * jax.nn.sigmoid(u @ Wg.T)
        gi = u @ W_ih.T                                        # (b,3H)
        gh = prev @ W_hh.T                                     # (b,3H)
        r = jax.nn.sigmoid(gi[:, :h] + gh[:, :h])
        z = jax.nn.sigmoid(gi[:, h:2 * h] + gh[:, h:2 * h])
        n = jnp.tanh(gi[:, 2 * h:] + r * gh[:, 2 * h:])
        new = (1.0 - z) * n + z * prev
        return new, new

    prev0 = jnp.zeros((passage_repr.shape[1], h), dtype=passage_repr.dtype)
    _, result = jax.lax.scan(step, prev0, passage)             # (P,b,H)
    return result


_pmapped = None


def _get_pmapped():
    global _pmapped
    if _pmapped is None:
        _pmapped = jax.pmap(
            _forward,
            in_axes=(0, 0, None, None, None, None, None, None, None),
        )
    return _pmapped


def kernel(**inputs):
    passage_repr = np.asarray(inputs["passage_repr"], dtype=np.float32)   # (P,B,I)
    question_repr = np.asarray(inputs["question_repr"], dtype=np.float32)  # (Q,B,I)
    WuQ = np.asarray(inputs["WuQ"], dtype=np.float32)
    WuP = np.asarray(inputs["WuP"], dtype=np.float32)
    WvP = np.asarray(inputs["WvP"], dtype=np.float32)
    vT = np.asarray(inputs["vT"], dtype=np.float32)
    Wg = np.asarray(inputs["Wg"], dtype=np.float32)
    W_ih = np.asarray(inputs["W_ih"], dtype=np.float32)
    W_hh = np.asarray(inputs["W_hh"], dtype=np.float32)

    # Shard batch (axis 1) across the 8 cores: (P,B,I) -> (NC,P,BS,I)
    p_sh = np.stack(np.split(passage_repr, NC, axis=1), axis=0)
    q_sh = np.stack(np.split(question_repr, NC, axis=1), axis=0)

    try:
        out_sh = _get_pmapped()(p_sh, q_sh, WuQ, WuP, WvP, vT, Wg, W_ih, W_hh)
        out_sh = np.asarray(out_sh)                            # (NC,P,BS,H)
    except Exception:
        # CPU fallback — correctness over speed
        with jax.default_device(jax.devices("cpu")[0]):
            outs = [
                np.asarray(_forward(jnp.asarray(p_sh[i]), jnp.asarray(q_sh[i]),
                                    WuQ, WuP, WvP, vT, Wg, W_ih, W_hh))
                for i in range(NC)
            ]
        out_sh = np.stack(outs, axis=0)

    # Unshard: (NC,P,BS,H) -> (P,B,H)
    return np.concatenate(list(out_sh), axis=1).astype(np.float32)

